# revision 21
# baseline (speedup 1.0000x reference)
"""MultiHeadLatentAttention prefill kernel for 8 Trainium2 NeuronCores.

Sharding: batch x head-group. Core j handles batch j//4 and head-group j%4
(4 of 16 heads). Every core sees the full 2048-token sequence of its batch, so
causality is identical across cores and future key blocks are skipped
STATICALLY (no padded keys, no per-core masks). Each core computes a partial
output ctx_g @ W_out[rows of its 4 heads]; the host sums the 4 partials per
batch (row-parallel out-projection, reduction folded into the unshard).

Matmuls run in fp8e4m3 with DoubleRow perf mode (0.5 cycles/row) on the
big-contraction legs (latent/q/k/v projections, attn@v, out-proj) and bf16 for
the q@k scores (contraction 128). Weights are pre-scaled by powers of two into
fp8's normal range; compensation is folded into the PSUM->SBUF casts and the
exp() scale, so it is numerically exact. Softmax skips max-subtraction
(|scores| <= ~1.3) and gets denominators from a ones-column appended to V.
"""
import sys

sys.path.insert(0, "/opt/trn_rl_repo")

import numpy as np
import ml_dtypes

import concourse.bass as bass
import concourse.bacc as bacc
import concourse.mybir as mybir
import concourse.tile as tile
from concourse import bass_utils
from concourse.masks import make_identity

# ---- config ---------------------------------------------------------------
FP8_PROJ = False   # x/wd/wq/wuk/wuv legs in fp8 + DoubleRow
FP8_ATTNV = False  # e/v in fp8 + DoubleRow attn@v
FP8_OUT = False    # ctx/wo in fp8 + DoubleRow out-proj
OUT_BF16 = True    # DMA partial outputs as bf16 (host upcasts + sums)

BF16 = ml_dtypes.bfloat16
F8 = ml_dtypes.float8_e4m3

B, T, D = 2, 2048, 2048
H, HD, L = 16, 128, 256          # total heads; per-core group of 4
HG = 4                            # heads per core
N_CORES = 8
NSTRIP = T // 128                 # 16 key strips
NCHUNK = T // 512                 # 4 T-chunks
SCALE = 1.0 / np.sqrt(HD)

F32 = mybir.dt.float32
DT_BF = mybir.dt.bfloat16
DT_F8 = mybir.dt.float8e4
DR = mybir.MatmulPerfMode.DoubleRow

DT_X = DT_F8 if FP8_PROJ else DT_BF
DT_PW = DT_F8 if FP8_PROJ else DT_BF      # wd/wq/wuk/wuv + latT storage
DT_E = DT_F8 if FP8_ATTNV else DT_BF      # exp(scores) + v storage
DT_C = DT_F8 if FP8_OUT else DT_BF        # ctx + wo
DT_O = DT_BF if OUT_BF16 else F32

# host-side pow2 weight scales (into fp8 normal range); 1.0 when bf16
SW = 2.0 ** 13 if FP8_PROJ else 1.0       # wq, wd
SUW = 2.0 ** 11 if FP8_PROJ else 1.0      # wuk, wuv
SOW = 2.0 ** 13 if FP8_OUT else 1.0       # wo


def _build_module():
    nc = bacc.Bacc("TRN2", target_bir_lowering=False, debug=False)

    xb_d = nc.dram_tensor("xb", [T, D], DT_X, kind="ExternalInput")
    wq_d = nc.dram_tensor("wq", [D, HG * HD], DT_PW, kind="ExternalInput")
    wd_d = nc.dram_tensor("wd", [D, L], DT_PW, kind="ExternalInput")
    wuk_d = nc.dram_tensor("wuk", [L, HG * HD], DT_PW, kind="ExternalInput")
    wuv_d = nc.dram_tensor("wuv", [L, HG * HD], DT_PW, kind="ExternalInput")
    wo_d = nc.dram_tensor("wo", [HG * HD, D], DT_C, kind="ExternalInput")
    tri_d = nc.dram_tensor("tri", [4, 128, 512], DT_E, kind="ExternalInput")
    out_d = nc.dram_tensor("out", [T, D], DT_O, kind="ExternalOutput")

    with tile.TileContext(nc) as tc:
        with (
            tc.tile_pool(name="const", bufs=1) as pconst,
            tc.tile_pool(name="ps", bufs=4, space="PSUM") as pps,
            tc.tile_pool(name="ctxps", bufs=2, space="PSUM") as pctx,
            tc.tile_pool(name="denps", bufs=2, space="PSUM") as pden,
        ):
            # ---- constants -------------------------------------------------
            ident = pconst.tile([128, 128], DT_BF)
            make_identity(nc, ident[:])
            # stationaries for denominator / reciprocal-broadcast matmuls
            ones_sb = pconst.tile([128, 1], DT_E)
            nc.gpsimd.memset(ones_sb[:], 1.0)
            onesr_f32 = pconst.tile([1, 128], F32)
            nc.gpsimd.memset(onesr_f32[:], 1.0)
            onesr_sb = onesr_f32[:].bitcast(mybir.dt.float32r)

            warm_sb = pconst.tile([128, 128], F32)
            wps = pps.tile([128, 128], F32, tag="ps", name="warm_ps")
            for i in range(72):
                nc.tensor.matmul(
                    wps[:], ident[:], ident[:], start=(i == 0), stop=(i == 71)
                )
            nc.vector.tensor_copy(warm_sb[:], wps[:])

            # persistent activations
            latT = pconst.tile([128, 2, T], DT_PW)       # (L-in-tile, lt, T)
            qT = pconst.tile([128, HG, T], DT_BF)        # (hd, head, T)
            kT = pconst.tile([128, HG, T], DT_BF)
            # v: (key-in-strip, strip, head, hd)
            v_sb = pconst.tile([128, NSTRIP, HG, HD], DT_E)
            ctxT = pconst.tile([128, HG // 2, 2, T], DT_C)  # (hd, hpair, elem, q)
            tri_sb = pconst.tile([128, 4, 512], DT_E)

            # reciprocal-scale const for v cast (DVE tensor_scalar needs an AP)
            rv_sb = pconst.tile([128, 1], F32)
            nc.gpsimd.memset(rv_sb[:], 1.0 / SUW)

            with (
                tc.tile_pool(name="xt", bufs=1) as pxt,
                tc.tile_pool(name="etile", bufs=6) as pe,
                tc.tile_pool(name="osb", bufs=2) as posb,
            ):
                wd_sb = pxt.tile([128, 8, 2, L], DT_PW)
                wq_sb = pxt.tile([128, 8, 2, HG * HD], DT_PW)
                wuk_sb = pxt.tile([128, 2, HG * HD], DT_PW)
                wuv_sb = pxt.tile([128, 2, HG * HD], DT_PW)
                xT_c = []
                for sg in range(NCHUNK):
                    xt = pxt.tile([128, 16, 512], DT_X, tag="xt", bufs=3,
                                  name=f"xT_{sg}")
                    xT_c.append(xt)
                wo_sb = pxt.tile([128, 2, 2, D], DT_C)

                # x^T chunk 0 first: the PE prologue depends on it
                nc.sync.dma_start_transpose(xT_c[0][:], xb_d.ap()[0:512, :])
                nc.sync.dma_start(
                    wd_sb[:],
                    wd_d.ap().rearrange("(dp two p) c -> p dp two c", p=128, two=2),
                )
                nc.sync.dma_start(
                    wq_sb[:],
                    wq_d.ap().rearrange("(dp two p) c -> p dp two c", p=128, two=2),
                )
                nc.sync.dma_start(
                    wuk_sb[:],
                    wuk_d.ap().rearrange("(two p) c -> p two c", p=128),
                )
                nc.sync.dma_start(
                    wuv_sb[:],
                    wuv_d.ap().rearrange("(two p) c -> p two c", p=128),
                )
                for ks in range(4):
                    nc.sync.dma_start(tri_sb[:, ks, :], tri_d.ap()[ks])
                nc.sync.dma_start_transpose(xT_c[1][:], xb_d.ap()[512:1024, :])
                nc.sync.dma_start(
                    wo_sb[:],
                    wo_d.ap().rearrange("(hp two p) c -> p hp two c", p=128, two=2),
                )
                nc.sync.dma_start_transpose(xT_c[2][:], xb_d.ap()[1024:1536, :])
                nc.sync.dma_start_transpose(xT_c[3][:], xb_d.ap()[1536:2048, :])

                def proj_chunk(sg):
                    c0 = sg * 512
                    xT = xT_c[sg]
                    # latent for T-chunk sg
                    for lt in range(2):
                        ps = pps.tile([128, 512], F32, tag="ps",
                                      name=f"lat_{sg}_{lt}")
                        if FP8_PROJ:
                            for dp in range(8):
                                nc.tensor.matmul(
                                    ps[:],
                                    wd_sb[:, dp, :, lt * 128 : (lt + 1) * 128],
                                    xT[:, 2 * dp : 2 * dp + 2, :],
                                    start=(dp == 0), stop=(dp == 7),
                                    perf_mode=DR,
                                )
                        else:
                            for dt in range(16):
                                nc.tensor.matmul(
                                    ps[:],
                                    wd_sb[:, dt // 2, dt % 2,
                                          lt * 128 : (lt + 1) * 128],
                                    xT[:, dt, :],
                                    start=(dt == 0), stop=(dt == 15),
                                )
                        nc.scalar.mul(latT[:, lt, c0 : c0 + 512], ps[:], 1.0 / SW)
                    # q^T for T-chunk sg, 4 heads
                    for h in range(HG):
                        ps = pps.tile([128, 512], F32, tag="ps",
                                      name=f"q_{sg}_{h}")
                        if FP8_PROJ:
                            for dp in range(8):
                                nc.tensor.matmul(
                                    ps[:],
                                    wq_sb[:, dp, :, h * 128 : (h + 1) * 128],
                                    xT[:, 2 * dp : 2 * dp + 2, :],
                                    start=(dp == 0), stop=(dp == 7),
                                    perf_mode=DR,
                                )
                        else:
                            for dt in range(16):
                                nc.tensor.matmul(
                                    ps[:],
                                    wq_sb[:, dt // 2, dt % 2,
                                          h * 128 : (h + 1) * 128],
                                    xT[:, dt, :],
                                    start=(dt == 0), stop=(dt == 15),
                                )
                        nc.scalar.mul(qT[:, h, c0 : c0 + 512], ps[:], 1.0 / SW)
                    # k^T for key-chunk sg, 4 heads (needs latT chunk sg)
                    for h in range(HG):
                        ps = pps.tile([128, 512], F32, tag="ps",
                                      name=f"k_{sg}_{h}")
                        if FP8_PROJ:
                            nc.tensor.matmul(
                                ps[:],
                                wuk_sb[:, :, h * 128 : (h + 1) * 128],
                                latT[:, :, c0 : c0 + 512],
                                start=True, stop=True, perf_mode=DR,
                            )
                        else:
                            for j in range(2):
                                nc.tensor.matmul(
                                    ps[:],
                                    wuk_sb[:, j, h * 128 : (h + 1) * 128],
                                    latT[:, j, c0 : c0 + 512],
                                    start=(j == 0), stop=(j == 1),
                                )
                        nc.scalar.mul(kT[:, h, c0 : c0 + 512], ps[:], 1.0 / SUW)
                    # v for key strips of chunk sg
                    for si in range(4):
                        s = sg * 4 + si
                        ps = pps.tile([128, 512], F32, tag="ps",
                                      name=f"v_{sg}_{si}")
                        if FP8_PROJ:
                            nc.tensor.matmul(
                                ps[:],
                                latT[:, :, s * 128 : (s + 1) * 128],
                                wuv_sb[:],
                                start=True, stop=True, perf_mode=DR,
                            )
                        else:
                            for j in range(2):
                                nc.tensor.matmul(
                                    ps[:],
                                    latT[:, j, s * 128 : (s + 1) * 128],
                                    wuv_sb[:, j, :],
                                    start=(j == 0), stop=(j == 1),
                                )
                        dst = v_sb[:, s, :, :]
                        nc.vector.tensor_scalar_mul(
                            dst, ps[:].rearrange("p (g c) -> p g c", c=HD),
                            rv_sb[:],
                        )

                cast_eng = [nc.scalar.copy, nc.vector.tensor_copy]

                def attn_chunk(qc):
                    q0 = qc * 512
                    nks = (qc + 1) * 4
                    for h in range(HG):
                        ctxp = pctx.tile([128, 512], F32, tag="ctx",
                                         name=f"ctx_{qc}_{h}")
                        denp = pden.tile([1, 512], F32, tag="den",
                                         name=f"den_{qc}_{h}")
                        e_tiles = [None] * nks

                        def emit_score(ks):
                            sps = pps.tile([128, 512], F32, tag="ps",
                                           name=f"s_{qc}_{h}_{ks}")
                            nc.tensor.matmul(
                                sps[:],
                                kT[:, h, ks * 128 : (ks + 1) * 128],
                                qT[:, h, q0 : q0 + 512],
                                start=True, stop=True,
                            )
                            e_s = pe.tile([128, 512], DT_E, tag="e",
                                          name=f"e_{qc}_{h}_{ks}")
                            nc.scalar.activation(
                                e_s[:], sps[:],
                                mybir.ActivationFunctionType.Exp,
                                scale=float(SCALE),
                            )
                            if ks >= 4 * qc:
                                nc.vector.tensor_mul(
                                    e_s[:], e_s[:], tri_sb[:, ks - 4 * qc, :]
                                )
                            e_tiles[ks] = e_s

                        emit_score(0)
                        if nks > 1:
                            emit_score(1)
                        for ks in range(nks):
                            if ks + 2 < nks:
                                emit_score(ks + 2)
                            e_s = e_tiles[ks]
                            nc.tensor.matmul(
                                ctxp[:], v_sb[:, ks, h, :], e_s[:],
                                start=(ks == 0), stop=(ks == nks - 1),
                            )
                            nc.tensor.matmul(
                                denp[:], ones_sb[:], e_s[:],
                                start=(ks == 0), stop=(ks == nks - 1),
                            )
                        # normalize: ctxT[:, :, q] = ctxp * (1/den) broadcast
                        rec = pe.tile([1, 512], mybir.dt.float32r, tag="rec",
                                      bufs=2)
                        with nc.allow_low_precision(
                            reason="f32r is fp32-width; bcast matmul needs f32r"
                        ):
                            nc.vector.reciprocal(rec[:], denp[:])
                        recb = pps.tile([128, 512], F32, tag="ps",
                                        name=f"rb_{qc}_{h}")
                        nc.tensor.matmul(
                            recb[:], onesr_sb, rec[:], start=True, stop=True
                        )
                        recb_sb = pe.tile([128, 512], F32, tag="recb", bufs=2)
                        nc.scalar.copy(recb_sb[:], recb[:])
                        nc.vector.tensor_mul(
                            ctxT[:, h // 2, h % 2, q0 : q0 + 512],
                            ctxp[:], recb_sb[:],
                        )

                def outproj_chunk(qc):
                    for i in range(4):
                        qs = 4 * qc + i
                        o_sb = posb.tile([128, 4, 512], DT_O, tag="o",
                                         name=f"o_{qc}_{i}")
                        for cc in range(4):
                            ops = pps.tile([128, 512], F32, tag="ps",
                                           name=f"op_{qs}_{cc}")
                            if FP8_OUT:
                                for hp in range(2):
                                    nc.tensor.matmul(
                                        ops[:],
                                        ctxT[:, hp, :, qs * 128 : (qs + 1) * 128],
                                        wo_sb[:, hp, :, cc * 512 : (cc + 1) * 512],
                                        start=(hp == 0), stop=(hp == 1),
                                        perf_mode=DR,
                                    )
                            else:
                                for hp in range(2):
                                    for j in range(2):
                                        nc.tensor.matmul(
                                            ops[:],
                                            ctxT[:, hp, j,
                                                 qs * 128 : (qs + 1) * 128],
                                            wo_sb[:, hp, j,
                                                  cc * 512 : (cc + 1) * 512],
                                            start=(hp == 0 and j == 0),
                                            stop=(hp == 1 and j == 1),
                                        )
                            if SOW != 1.0:
                                nc.scalar.mul(o_sb[:, cc, :], ops[:], 1.0 / SOW)
                            else:
                                cast_eng[cc % 2](o_sb[:, cc, :], ops[:])
                            nc.sync.dma_start(
                                out_d.ap()[qs * 128 : (qs + 1) * 128,
                                           cc * 512 : (cc + 1) * 512],
                                o_sb[:, cc, :],
                            )

                for sg in range(NCHUNK):
                    proj_chunk(sg)
                    attn_chunk(sg)
                    outproj_chunk(sg)

    nc.compile()
    return nc


_NC_CACHE = None


def _get_module():
    global _NC_CACHE
    if _NC_CACHE is None:
        _NC_CACHE = _build_module()
    return _NC_CACHE


def _np_dt(dt):
    return {DT_BF: BF16, DT_F8: F8, F32: np.float32}[dt]


def _host_prep(x, W_query, W_down, W_up_k, W_up_v, W_out):
    xc = [np.ascontiguousarray(x[b]).astype(_np_dt(DT_X)) for b in range(B)]
    wd = (W_down * SW).astype(_np_dt(DT_PW))
    wq_g = [
        np.ascontiguousarray(W_query[:, g * 512 : (g + 1) * 512] * SW)
        .astype(_np_dt(DT_PW))
        for g in range(4)
    ]
    wuk_g = [
        np.ascontiguousarray(W_up_k[:, g * 512 : (g + 1) * 512] * SUW)
        .astype(_np_dt(DT_PW))
        for g in range(4)
    ]
    wuv_g = [
        np.ascontiguousarray(W_up_v[:, g * 512 : (g + 1) * 512] * SUW)
        .astype(_np_dt(DT_PW))
        for g in range(4)
    ]
    wo_g = [
        np.ascontiguousarray(W_out[g * 512 : (g + 1) * 512, :] * SOW)
        .astype(_np_dt(DT_C))
        for g in range(4)
    ]
    kk = np.arange(512).reshape(4, 128, 1)
    qq = np.arange(512).reshape(1, 1, 512)
    tri = (kk <= qq).astype(_np_dt(DT_E))

    in_maps = []
    for j in range(N_CORES):
        b, g = divmod(j, 4)
        in_maps.append(
            {"xb": xc[b], "wq": wq_g[g], "wd": wd, "wuk": wuk_g[g],
             "wuv": wuv_g[g], "wo": wo_g[g], "tri": tri}
        )
    return in_maps


def kernel(x, W_query, W_down, W_up_k, W_up_v, W_out, _trace=False, _trace_kwargs=None):
    x = np.asarray(x, dtype=np.float32)
    in_maps = _host_prep(
        x,
        np.asarray(W_query, np.float32),
        np.asarray(W_down, np.float32),
        np.asarray(W_up_k, np.float32),
        np.asarray(W_up_v, np.float32),
        np.asarray(W_out, np.float32),
    )
    nc = _get_module()
    res = bass_utils.run_bass_kernel_spmd(
        nc, in_maps, core_ids=list(range(N_CORES)), trace=_trace,
        **(_trace_kwargs or {}),
    )
    y = np.zeros((B, T, D), np.float32)
    for j in range(N_CORES):
        b, g = divmod(j, 4)
        y[b] += res.results[j]["out"].astype(np.float32)
    kernel._last_results = res
    return y


# revision 22
# speedup vs baseline: 1.4590x; 1.4590x over previous
"""MultiHeadLatentAttention prefill kernel for 8 Trainium2 NeuronCores.

Sharding: batch x head-group. Core j handles batch j//4 and head-group j%4
(4 of 16 heads). Every core sees the full 2048-token sequence of its batch, so
causality is identical across cores and future key blocks are skipped
STATICALLY (no padded keys, no per-core masks). Each core computes a partial
output ctx_g @ W_out[rows of its 4 heads]; the host sums the 4 partials per
batch (row-parallel out-projection, reduction folded into the unshard).

Matmuls run in fp8e4m3 with DoubleRow perf mode (0.5 cycles/row) on the
big-contraction legs (latent/q/k/v projections, attn@v, out-proj) and bf16 for
the q@k scores (contraction 128). Weights are pre-scaled by powers of two into
fp8's normal range; compensation is folded into the PSUM->SBUF casts and the
exp() scale, so it is numerically exact. Softmax skips max-subtraction
(|scores| <= ~1.3) and gets denominators from a ones-column appended to V.
"""
import sys

sys.path.insert(0, "/opt/trn_rl_repo")

import numpy as np
import ml_dtypes

import concourse.bass as bass
import concourse.bacc as bacc
import concourse.mybir as mybir
import concourse.tile as tile
from concourse import bass_utils
from concourse.masks import make_identity

# ---- config ---------------------------------------------------------------
FP8_PROJ = False   # x/wd/wq/wuk/wuv legs in fp8 + DoubleRow
FP8_ATTNV = False  # e/v in fp8 + DoubleRow attn@v
FP8_OUT = False    # ctx/wo in fp8 + DoubleRow out-proj
OUT_BF16 = True    # DMA partial outputs as bf16 (host upcasts + sums)

BF16 = ml_dtypes.bfloat16
F8 = ml_dtypes.float8_e4m3

B, T, D = 2, 2048, 2048
H, HD, L = 16, 128, 256          # total heads; per-core group of 4
HG = 4                            # heads per core
N_CORES = 8
NSTRIP = T // 128                 # 16 key strips
NCHUNK = T // 512                 # 4 T-chunks
SCALE = 1.0 / np.sqrt(HD)

F32 = mybir.dt.float32
DT_BF = mybir.dt.bfloat16
DT_F8 = mybir.dt.float8e4
DR = mybir.MatmulPerfMode.DoubleRow

DT_X = DT_F8 if FP8_PROJ else DT_BF
DT_PW = DT_F8 if FP8_PROJ else DT_BF      # wd/wq/wuk/wuv + latT storage
DT_E = DT_F8 if FP8_ATTNV else DT_BF      # exp(scores) + v storage
DT_C = DT_F8 if FP8_OUT else DT_BF        # ctx + wo
DT_O = DT_BF if OUT_BF16 else F32

# host-side pow2 weight scales (into fp8 normal range); 1.0 when bf16
SW = 2.0 ** 13 if FP8_PROJ else 1.0       # wq, wd
SUW = 2.0 ** 11 if FP8_PROJ else 1.0      # wuk, wuv
SOW = 2.0 ** 13 if FP8_OUT else 1.0       # wo


def _build_module():
    nc = bacc.Bacc("TRN2", target_bir_lowering=False, debug=False)

    xb_d = nc.dram_tensor("xb", [T, D], DT_X, kind="ExternalInput")
    wq_d = nc.dram_tensor("wq", [D, HG * HD], DT_PW, kind="ExternalInput")
    wd_d = nc.dram_tensor("wd", [D, L], DT_PW, kind="ExternalInput")
    wuk_d = nc.dram_tensor("wuk", [L, HG * HD], DT_PW, kind="ExternalInput")
    wuv_d = nc.dram_tensor("wuv", [L, HG * HD], DT_PW, kind="ExternalInput")
    wo_d = nc.dram_tensor("wo", [HG * HD, D], DT_C, kind="ExternalInput")
    tri_d = nc.dram_tensor("tri", [4, 128, 512], DT_E, kind="ExternalInput")
    out_d = nc.dram_tensor("out", [T, D], DT_O, kind="ExternalOutput")

    with tile.TileContext(nc) as tc:
        with (
            tc.tile_pool(name="const", bufs=1) as pconst,
            tc.tile_pool(name="ps", bufs=3, space="PSUM") as pps,
            tc.tile_pool(name="ctxps", bufs=4, space="PSUM") as pctx,
            tc.tile_pool(name="tps", bufs=1, space="PSUM") as ptp,
        ):
            # ---- constants -------------------------------------------------
            ident = pconst.tile([128, 128], DT_BF)
            make_identity(nc, ident[:])

            warm_sb = pconst.tile([128, 128], F32)
            wps = pps.tile([128, 128], F32, tag="ps", name="warm_ps")
            for i in range(72):
                nc.tensor.matmul(
                    wps[:], ident[:], ident[:], start=(i == 0), stop=(i == 71)
                )
            nc.vector.tensor_copy(warm_sb[:], wps[:])

            # persistent activations
            latT = pconst.tile([128, 2, T], DT_PW)       # (L-in-tile, lt, T)
            qT = pconst.tile([128, HG, T], DT_BF)        # (hd, head, T)
            kT = pconst.tile([128, HG, T], DT_BF)
            # v: (key-in-strip, strip, head, hd + ones-column)
            v_sb = pconst.tile([128, NSTRIP, HG, HD + 1], DT_E)
            ctxT = pconst.tile([128, HG // 2, 2, T], DT_C)  # (hd, hpair, elem, q)
            tri_sb = pconst.tile([128, 4, 512], DT_E)

            nc.gpsimd.memset(v_sb[:, :, :, HD : HD + 1], 1.0)

            # reciprocal-scale const for v cast (DVE tensor_scalar needs an AP)
            rv_sb = pconst.tile([128, 1], F32)
            nc.gpsimd.memset(rv_sb[:], 1.0 / SUW)

            with (
                tc.tile_pool(name="xt", bufs=1) as pxt,
                tc.tile_pool(name="etile", bufs=6) as pe,
                tc.tile_pool(name="osb", bufs=2) as posb,
            ):
                wd_sb = pxt.tile([128, 8, 2, L], DT_PW)
                wq_sb = pxt.tile([128, 8, 2, HG * HD], DT_PW)
                wuk_sb = pxt.tile([128, 2, HG * HD], DT_PW)
                wuv_sb = pxt.tile([128, 2, HG * HD], DT_PW)
                xT_c = []
                for sg in range(NCHUNK):
                    xt = pxt.tile([128, 16, 512], DT_X, tag="xt", bufs=3,
                                  name=f"xT_{sg}")
                    xT_c.append(xt)
                wo_sb = pxt.tile([128, 2, 2, D], DT_C)

                # x^T chunk 0 first: the PE prologue depends on it
                nc.sync.dma_start_transpose(xT_c[0][:], xb_d.ap()[0:512, :])
                nc.sync.dma_start(
                    wd_sb[:],
                    wd_d.ap().rearrange("(dp two p) c -> p dp two c", p=128, two=2),
                )
                nc.sync.dma_start(
                    wq_sb[:],
                    wq_d.ap().rearrange("(dp two p) c -> p dp two c", p=128, two=2),
                )
                nc.sync.dma_start(
                    wuk_sb[:],
                    wuk_d.ap().rearrange("(two p) c -> p two c", p=128),
                )
                nc.sync.dma_start(
                    wuv_sb[:],
                    wuv_d.ap().rearrange("(two p) c -> p two c", p=128),
                )
                for ks in range(4):
                    nc.sync.dma_start(tri_sb[:, ks, :], tri_d.ap()[ks])
                nc.sync.dma_start_transpose(xT_c[1][:], xb_d.ap()[512:1024, :])
                nc.sync.dma_start(
                    wo_sb[:],
                    wo_d.ap().rearrange("(hp two p) c -> p hp two c", p=128, two=2),
                )
                nc.sync.dma_start_transpose(xT_c[2][:], xb_d.ap()[1024:1536, :])
                nc.sync.dma_start_transpose(xT_c[3][:], xb_d.ap()[1536:2048, :])

                def proj_chunk(sg):
                    c0 = sg * 512
                    xT = xT_c[sg]
                    # latent for T-chunk sg
                    for lt in range(2):
                        ps = pps.tile([128, 512], F32, tag="ps",
                                      name=f"lat_{sg}_{lt}")
                        if FP8_PROJ:
                            for dp in range(8):
                                nc.tensor.matmul(
                                    ps[:],
                                    wd_sb[:, dp, :, lt * 128 : (lt + 1) * 128],
                                    xT[:, 2 * dp : 2 * dp + 2, :],
                                    start=(dp == 0), stop=(dp == 7),
                                    perf_mode=DR,
                                )
                        else:
                            for dt in range(16):
                                nc.tensor.matmul(
                                    ps[:],
                                    wd_sb[:, dt // 2, dt % 2,
                                          lt * 128 : (lt + 1) * 128],
                                    xT[:, dt, :],
                                    start=(dt == 0), stop=(dt == 15),
                                )
                        nc.scalar.mul(latT[:, lt, c0 : c0 + 512], ps[:], 1.0 / SW)
                    # q^T for T-chunk sg, 4 heads
                    for h in range(HG):
                        ps = pps.tile([128, 512], F32, tag="ps",
                                      name=f"q_{sg}_{h}")
                        if FP8_PROJ:
                            for dp in range(8):
                                nc.tensor.matmul(
                                    ps[:],
                                    wq_sb[:, dp, :, h * 128 : (h + 1) * 128],
                                    xT[:, 2 * dp : 2 * dp + 2, :],
                                    start=(dp == 0), stop=(dp == 7),
                                    perf_mode=DR,
                                )
                        else:
                            for dt in range(16):
                                nc.tensor.matmul(
                                    ps[:],
                                    wq_sb[:, dt // 2, dt % 2,
                                          h * 128 : (h + 1) * 128],
                                    xT[:, dt, :],
                                    start=(dt == 0), stop=(dt == 15),
                                )
                        nc.scalar.mul(qT[:, h, c0 : c0 + 512], ps[:], 1.0 / SW)
                    # k^T for key-chunk sg, 4 heads (needs latT chunk sg)
                    for h in range(HG):
                        ps = pps.tile([128, 512], F32, tag="ps",
                                      name=f"k_{sg}_{h}")
                        if FP8_PROJ:
                            nc.tensor.matmul(
                                ps[:],
                                wuk_sb[:, :, h * 128 : (h + 1) * 128],
                                latT[:, :, c0 : c0 + 512],
                                start=True, stop=True, perf_mode=DR,
                            )
                        else:
                            for j in range(2):
                                nc.tensor.matmul(
                                    ps[:],
                                    wuk_sb[:, j, h * 128 : (h + 1) * 128],
                                    latT[:, j, c0 : c0 + 512],
                                    start=(j == 0), stop=(j == 1),
                                )
                        nc.scalar.mul(kT[:, h, c0 : c0 + 512], ps[:], 1.0 / SUW)
                    # v for key strips of chunk sg
                    for si in range(4):
                        s = sg * 4 + si
                        ps = pps.tile([128, 512], F32, tag="ps",
                                      name=f"v_{sg}_{si}")
                        if FP8_PROJ:
                            nc.tensor.matmul(
                                ps[:],
                                latT[:, :, s * 128 : (s + 1) * 128],
                                wuv_sb[:],
                                start=True, stop=True, perf_mode=DR,
                            )
                        else:
                            for j in range(2):
                                nc.tensor.matmul(
                                    ps[:],
                                    latT[:, j, s * 128 : (s + 1) * 128],
                                    wuv_sb[:, j, :],
                                    start=(j == 0), stop=(j == 1),
                                )
                        dst = v_sb[:, s, :, :HD]
                        nc.vector.tensor_scalar_mul(
                            dst, ps[:].rearrange("p (g c) -> p g c", c=HD),
                            rv_sb[:],
                        )

                cast_eng = [nc.scalar.copy, nc.vector.tensor_copy]

                def attn_chunk(qc):
                    q0 = qc * 512
                    nks = (qc + 1) * 4
                    for h in range(HG):
                        ctx_ps = [
                            pctx.tile([128, HD + 1], F32, tag="ctx",
                                      name=f"ctx_{qc}_{h}_{i}")
                            for i in range(4)
                        ]
                        e_tiles = [None] * nks

                        def emit_score(ks):
                            sps = pps.tile([128, 512], F32, tag="ps",
                                           name=f"s_{qc}_{h}_{ks}")
                            nc.tensor.matmul(
                                sps[:],
                                kT[:, h, ks * 128 : (ks + 1) * 128],
                                qT[:, h, q0 : q0 + 512],
                                start=True, stop=True,
                            )
                            e_s = pe.tile([128, 512], DT_E, tag="e",
                                          name=f"e_{qc}_{h}_{ks}")
                            nc.scalar.activation(
                                e_s[:], sps[:],
                                mybir.ActivationFunctionType.Exp,
                                scale=float(SCALE),
                            )
                            if ks >= 4 * qc:
                                nc.vector.tensor_mul(
                                    e_s[:], e_s[:], tri_sb[:, ks - 4 * qc, :]
                                )
                            e_tiles[ks] = e_s

                        emit_score(0)
                        if nks > 1:
                            emit_score(1)
                        for ks in range(nks):
                            if ks + 2 < nks:
                                emit_score(ks + 2)
                            e_s = e_tiles[ks]
                            for i in range(4):
                                qs = 4 * qc + i
                                if ks > qs:
                                    continue
                                nc.tensor.matmul(
                                    ctx_ps[i][:],
                                    e_s[:, i * 128 : (i + 1) * 128],
                                    v_sb[:, ks, h, :],
                                    start=(ks == 0), stop=(ks == qs),
                                )
                        # normalize + transpose into out-proj lhsT layout
                        for i in range(4):
                            qs = 4 * qc + i
                            rec = pe.tile([128, 1], F32, tag="rec", bufs=2)
                            nc.vector.reciprocal(rec[:], ctx_ps[i][:, HD : HD + 1])
                            ctxn = pe.tile([128, HD], DT_C, tag="ctxn")
                            nc.vector.tensor_scalar_mul(
                                ctxn[:], ctx_ps[i][:, :HD], rec[:]
                            )
                            tps = ptp.tile([128, 128], DT_C, tag="ctp", bufs=1,
                                           name=f"ct_{qc}_{h}_{i}")
                            nc.tensor.transpose(tps[:], ctxn[:], ident[:])
                            nc.vector.tensor_copy(
                                ctxT[:, h // 2, h % 2,
                                     qs * 128 : (qs + 1) * 128],
                                tps[:],
                            )

                def outproj_chunk(qc):
                    for i in range(4):
                        qs = 4 * qc + i
                        o_sb = posb.tile([128, 4, 512], DT_O, tag="o",
                                         name=f"o_{qc}_{i}")
                        for cc in range(4):
                            ops = pps.tile([128, 512], F32, tag="ps",
                                           name=f"op_{qs}_{cc}")
                            if FP8_OUT:
                                for hp in range(2):
                                    nc.tensor.matmul(
                                        ops[:],
                                        ctxT[:, hp, :, qs * 128 : (qs + 1) * 128],
                                        wo_sb[:, hp, :, cc * 512 : (cc + 1) * 512],
                                        start=(hp == 0), stop=(hp == 1),
                                        perf_mode=DR,
                                    )
                            else:
                                for hp in range(2):
                                    for j in range(2):
                                        nc.tensor.matmul(
                                            ops[:],
                                            ctxT[:, hp, j,
                                                 qs * 128 : (qs + 1) * 128],
                                            wo_sb[:, hp, j,
                                                  cc * 512 : (cc + 1) * 512],
                                            start=(hp == 0 and j == 0),
                                            stop=(hp == 1 and j == 1),
                                        )
                            if SOW != 1.0:
                                nc.scalar.mul(o_sb[:, cc, :], ops[:], 1.0 / SOW)
                            else:
                                cast_eng[cc % 2](o_sb[:, cc, :], ops[:])
                            nc.sync.dma_start(
                                out_d.ap()[qs * 128 : (qs + 1) * 128,
                                           cc * 512 : (cc + 1) * 512],
                                o_sb[:, cc, :],
                            )

                for sg in range(NCHUNK):
                    proj_chunk(sg)
                    attn_chunk(sg)
                    outproj_chunk(sg)

    nc.compile()
    return nc


_NC_CACHE = None


def _get_module():
    global _NC_CACHE
    if _NC_CACHE is None:
        _NC_CACHE = _build_module()
    return _NC_CACHE


def _np_dt(dt):
    return {DT_BF: BF16, DT_F8: F8, F32: np.float32}[dt]


def _host_prep(x, W_query, W_down, W_up_k, W_up_v, W_out):
    xc = [np.ascontiguousarray(x[b]).astype(_np_dt(DT_X)) for b in range(B)]
    wd = (W_down * SW).astype(_np_dt(DT_PW))
    wq_g = [
        np.ascontiguousarray(W_query[:, g * 512 : (g + 1) * 512] * SW)
        .astype(_np_dt(DT_PW))
        for g in range(4)
    ]
    wuk_g = [
        np.ascontiguousarray(W_up_k[:, g * 512 : (g + 1) * 512] * SUW)
        .astype(_np_dt(DT_PW))
        for g in range(4)
    ]
    wuv_g = [
        np.ascontiguousarray(W_up_v[:, g * 512 : (g + 1) * 512] * SUW)
        .astype(_np_dt(DT_PW))
        for g in range(4)
    ]
    wo_g = [
        np.ascontiguousarray(W_out[g * 512 : (g + 1) * 512, :] * SOW)
        .astype(_np_dt(DT_C))
        for g in range(4)
    ]
    kk = np.arange(512).reshape(4, 128, 1)
    qq = np.arange(512).reshape(1, 1, 512)
    tri = (kk <= qq).astype(_np_dt(DT_E))

    in_maps = []
    for j in range(N_CORES):
        b, g = divmod(j, 4)
        in_maps.append(
            {"xb": xc[b], "wq": wq_g[g], "wd": wd, "wuk": wuk_g[g],
             "wuv": wuv_g[g], "wo": wo_g[g], "tri": tri}
        )
    return in_maps


def kernel(x, W_query, W_down, W_up_k, W_up_v, W_out, _trace=False, _trace_kwargs=None):
    x = np.asarray(x, dtype=np.float32)
    in_maps = _host_prep(
        x,
        np.asarray(W_query, np.float32),
        np.asarray(W_down, np.float32),
        np.asarray(W_up_k, np.float32),
        np.asarray(W_up_v, np.float32),
        np.asarray(W_out, np.float32),
    )
    nc = _get_module()
    res = bass_utils.run_bass_kernel_spmd(
        nc, in_maps, core_ids=list(range(N_CORES)), trace=_trace,
        **(_trace_kwargs or {}),
    )
    y = np.zeros((B, T, D), np.float32)
    for j in range(N_CORES):
        b, g = divmod(j, 4)
        y[b] += res.results[j]["out"].astype(np.float32)
    kernel._last_results = res
    return y
